# revision 4
# baseline (speedup 1.0000x reference)
"""Distributed Trainium2 Bass kernel for nn_AGCN (gnn_message_passing).

Reference computation (B=1, CHNN=1024, K=L=2048):
    vcw  = softmax_k(W_ak @ vc + b_ak)            # (K, L) assignment weights
    vmr  = relu(W_c @ vm + b_c)                   # (C, K)
    vma  = vmr @ vcw                              # (C, L)
    vmad = W_mad @ vma + b_mad                    # (C, L)
    A    = vmad^T @ vmad                          # (K, L) gram (symmetric)
    x    = vmr^T @ W_gcn + b_gcn                  # (K, C)
    out  = (softmax_rows(A) @ x)^T                # (C, L)

Distribution: position (node) sharding.  Core i owns 256 of the 2048 node
columns.  Everything is local except two bf16 AllGathers: vmrT (2048,1024)
and vmad (1024,256 per-core shards).  The final GEMM is refactored as
out = (A_sm @ vmr^T) @ W_gcn + b_gcn (rows of A_sm sum to 1, so the bias
add is exact), which removes a third all-gather.

All matmuls run in bf16 (fp32 PSUM accumulation).  Numerically validated:
full-bf16 pipeline gives ~9e-4 max rel err vs the fp64 reference (the A
softmax is near-uniform, so gram noise averages out).  Softmaxes skip
max-subtraction (z in ±3.4, A in [16.8, 17.2] for this input scale; exp
stays well inside fp32/bf16 range).
"""

import numpy as np
import ml_dtypes

import concourse.bass as bass
import concourse.mybir as mybir
import concourse.tile as tile
from concourse import bacc
from concourse import bass_utils

P = 128            # partitions
C = 1024           # channels (8 tiles)
K = 2048           # nodes (16 tiles)
NCORES = 8
KL = K // NCORES   # 256 local node columns per core
CT = C // P        # 8
KT = K // P        # 16
KLT = KL // P      # 2

BF = mybir.dt.bfloat16
F32 = mybir.dt.float32
RG = [list(range(NCORES))]

Exp = mybir.ActivationFunctionType.Exp
Identity = mybir.ActivationFunctionType.Identity


def build():
    nc = bacc.Bacc("TRN2", target_bir_lowering=False, debug=False,
                   num_devices=NCORES)

    # ---- kernel I/O (per-core) ----
    vc_i = nc.dram_tensor("vc_i", [C, KL], BF, kind="ExternalInput").ap()
    vm_i = nc.dram_tensor("vm_i", [C, KL], BF, kind="ExternalInput").ap()
    W_akT = nc.dram_tensor("W_akT", [C, K], BF, kind="ExternalInput").ap()
    W_cT = nc.dram_tensor("W_cT", [C, C], BF, kind="ExternalInput").ap()
    W_madT = nc.dram_tensor("W_madT", [C, C], BF, kind="ExternalInput").ap()
    W_gcn = nc.dram_tensor("W_gcn", [C, C], BF, kind="ExternalInput").ap()
    b_ak_t = nc.dram_tensor("b_ak_t", [P, KT], F32, kind="ExternalInput").ap()
    b_cB = nc.dram_tensor("b_cB", [P, C], F32, kind="ExternalInput").ap()
    b_mad_t = nc.dram_tensor("b_mad_t", [P, CT], F32, kind="ExternalInput").ap()
    b_gcn_t = nc.dram_tensor("b_gcn_t", [P, CT], F32, kind="ExternalInput").ap()
    out = nc.dram_tensor("out", [C, KL], F32, kind="ExternalOutput").ap()

    with tile.TileContext(nc) as tc:
        with (
            tc.tile_pool(name="const", bufs=1) as const,
            tc.tile_pool(name="stage", bufs=3) as stage,
            tc.tile_pool(name="psum", bufs=4, space="PSUM") as psum_pool,
            tc.tile_pool(name="cspsum", bufs=1, space="PSUM") as cs_pool,
            tc.tile_pool(name="dram", bufs=1, space="DRAM") as dram,
        ):
            # ---- persistent SBUF tensors ----
            vm_sb = const.tile([P, CT, KL], BF)       # vm   [p, ct, kl]
            vc_sb = const.tile([P, CT, KL], BF)
            WcT_sb = const.tile([P, CT, C], BF)       # W_c^T rows
            WakT_sb = const.tile([P, CT, K], BF)
            WmadT_sb = const.tile([P, CT, C], BF)
            Wgcn_sb = const.tile([P, CT, C], BF)
            vmrT_full = const.tile([P, KT, C], BF)    # vmr^T[kt*128+p, c]
            # vmad_full shares the WakT slot (same shape/dtype, disjoint lifetime)
            vmad_full = const.tile([P, CT, K], BF, tag="WakT_sb")
            exp_sb = const.tile([P, KT, KL], BF)      # expz then expA (reused)
            expA_sb = exp_sb                          # alias for readability
            vma_sb = const.tile([P, CT, KL], BF)
            vmad_i_sb = const.tile([P, CT, KL], BF)
            T1T_sb = const.tile([P, CT, KL], BF)
            b_ak_sb = const.tile([P, KT], F32)
            b_cB_sb = const.tile([P, C], F32)
            b_mad_sb = const.tile([P, CT], F32)
            b_gcn_sb = const.tile([P, CT], F32)
            onesm = const.tile([P, P], BF)            # 128x128 ones
            recipL = const.tile([P, KL], F32)         # 1/colsum(expz), bcast rows
            recipR = const.tile([P, KL], F32)

            nc.any.memset(onesm, 1.0)

            # ---- input DMA loads (per c-tile for pipelining) ----
            nc.sync.dma_start(out=b_ak_sb, in_=b_ak_t)
            nc.sync.dma_start(out=b_cB_sb, in_=b_cB)
            nc.sync.dma_start(out=b_mad_sb, in_=b_mad_t)
            nc.sync.dma_start(out=b_gcn_sb, in_=b_gcn_t)
            for ct in range(CT):
                rows = slice(ct * P, (ct + 1) * P)
                nc.sync.dma_start(out=vm_sb[:, ct, :], in_=vm_i[rows, :])
                nc.sync.dma_start(out=WcT_sb[:, ct, :], in_=W_cT[rows, :])
            for ct in range(CT):
                rows = slice(ct * P, (ct + 1) * P)
                nc.sync.dma_start(out=vc_sb[:, ct, :], in_=vc_i[rows, :])
                nc.sync.dma_start(out=WakT_sb[:, ct, :], in_=W_akT[rows, :])
            for ct in range(CT):
                rows = slice(ct * P, (ct + 1) * P)
                nc.sync.dma_start(out=WmadT_sb[:, ct, :], in_=W_madT[rows, :])
                nc.sync.dma_start(out=Wgcn_sb[:, ct, :], in_=W_gcn[rows, :])

            # ---- collective bounce buffers ----
            ag1_in = dram.tile([KL, C], BF)
            ag1_out = dram.tile([K, C], BF, addr_space="Shared")
            ag2_in = dram.tile([C, KL], BF)
            ag2_out = dram.tile([NCORES, C, KL], BF, addr_space="Shared")

            # ================= S1: vmrT_i = relu(vm_i^T W_c^T + b_c) ======
            # out (KL=256 rows, C=1024): 2 k-slices x 2 n-chunks of 512
            for kt in range(KLT):
                ksl = slice(kt * P, (kt + 1) * P)
                for n in range(2):
                    nsl = slice(n * 512, (n + 1) * 512)
                    ps = psum_pool.tile([P, 512], F32, tag="ps512", bufs=2)
                    for cc in range(CT):
                        nc.tensor.matmul(ps, vm_sb[:, cc, ksl],
                                         WcT_sb[:, cc, nsl],
                                         start=(cc == 0), stop=(cc == CT - 1))
                    tmp = stage.tile([P, 512], F32, tag="s1tmp")
                    nc.vector.tensor_add(tmp, ps, b_cB_sb[:, nsl])
                    relu = stage.tile([P, 512], BF, tag="s1relu")
                    nc.vector.tensor_scalar_max(relu, tmp, 0.0)
                    nc.sync.dma_start(out=ag1_in[ksl, nsl], in_=relu)

            # ================= AG1: all-gather vmrT ========================
            nc.gpsimd.collective_compute(
                "AllGather", mybir.AluOpType.bypass, replica_groups=RG,
                ins=[ag1_in.opt()], outs=[ag1_out.opt()],
            )

            # ================= S3: expz = exp(W_ak vc + b_ak) ==============
            # out (K=2048 rows, KL=256): 16 k-tiles; colsum on all partitions
            cs1 = cs_pool.tile([P, KL], F32, tag="cs")
            for kt in range(KT):
                ksl = slice(kt * P, (kt + 1) * P)
                ps = psum_pool.tile([P, KL], F32, tag="ps256")
                for cc in range(CT):
                    nc.tensor.matmul(ps, WakT_sb[:, cc, ksl], vc_sb[:, cc, :],
                                     start=(cc == 0), stop=(cc == CT - 1))
                nc.scalar.activation(exp_sb[:, kt, :], ps, Exp,
                                     bias=b_ak_sb[:, kt:kt + 1], scale=1.0)
                nc.tensor.matmul(cs1, onesm, exp_sb[:, kt, :],
                                 start=(kt == 0), stop=(kt == KT - 1))
            nc.vector.reciprocal(recipL, cs1)

            # ---- load gathered vmrT into SBUF ----
            for kt in range(KT):
                rows = slice(kt * P, (kt + 1) * P)
                nc.sync.dma_start(out=vmrT_full[:, kt, :], in_=ag1_out[rows, :])

            # ================= S4: vma = (vmr @ expz) * recipL =============
            for m in range(CT):
                msl = slice(m * P, (m + 1) * P)
                ps = psum_pool.tile([P, KL], F32, tag="ps256")
                for kt in range(KT):
                    nc.tensor.matmul(ps, vmrT_full[:, kt, msl],
                                     exp_sb[:, kt, :],
                                     start=(kt == 0), stop=(kt == KT - 1))
                nc.vector.tensor_mul(vma_sb[:, m, :], ps, recipL)

            # ================= S5: vmad = W_mad vma + b_mad ================
            for m in range(CT):
                msl = slice(m * P, (m + 1) * P)
                ps = psum_pool.tile([P, KL], F32, tag="ps256")
                for cc in range(CT):
                    nc.tensor.matmul(ps, WmadT_sb[:, cc, msl], vma_sb[:, cc, :],
                                     start=(cc == 0), stop=(cc == CT - 1))
                nc.scalar.activation(vmad_i_sb[:, m, :], ps, Identity,
                                     bias=b_mad_sb[:, m:m + 1], scale=1.0)
                nc.sync.dma_start(out=ag2_in[msl, :], in_=vmad_i_sb[:, m, :])

            # ================= AG2: all-gather vmad ========================
            nc.gpsimd.collective_compute(
                "AllGather", mybir.AluOpType.bypass, replica_groups=RG,
                ins=[ag2_in.opt()], outs=[ag2_out.opt()],
            )
            # reassemble (c, k) layout: dst[p, ct, s*KL+kl] = src[s, ct*128+p, kl]
            for ct in range(CT):
                rows = slice(ct * P, (ct + 1) * P)
                nc.sync.dma_start(
                    out=vmad_full[:, ct, :].rearrange("p (s k) -> p s k",
                                                      s=NCORES),
                    in_=ag2_out[:, rows, :].rearrange("s p k -> p s k"),
                )

            # ================= S7: expA = exp(vmad_full^T vmad_i) ==========
            cs2 = cs_pool.tile([P, KL], F32, tag="cs")
            for kt in range(KT):
                ksl = slice(kt * P, (kt + 1) * P)
                ps = psum_pool.tile([P, KL], F32, tag="ps256")
                for cc in range(CT):
                    nc.tensor.matmul(ps, vmad_full[:, cc, ksl],
                                     vmad_i_sb[:, cc, :],
                                     start=(cc == 0), stop=(cc == CT - 1))
                nc.scalar.activation(expA_sb[:, kt, :], ps, Exp)
                nc.tensor.matmul(cs2, onesm, expA_sb[:, kt, :],
                                 start=(kt == 0), stop=(kt == KT - 1))
            nc.vector.reciprocal(recipR, cs2)

            # ================= S8: T1T = (vmr @ expA) * recipR =============
            for m in range(CT):
                msl = slice(m * P, (m + 1) * P)
                ps = psum_pool.tile([P, KL], F32, tag="ps256")
                for kt in range(KT):
                    nc.tensor.matmul(ps, vmrT_full[:, kt, msl],
                                     expA_sb[:, kt, :],
                                     start=(kt == 0), stop=(kt == KT - 1))
                nc.vector.tensor_mul(T1T_sb[:, m, :], ps, recipR)

            # ================= S9: out = W_gcn^T T1T + b_gcn ===============
            for m in range(CT):
                msl = slice(m * P, (m + 1) * P)
                ps = psum_pool.tile([P, KL], F32, tag="ps256")
                for cc in range(CT):
                    nc.tensor.matmul(ps, Wgcn_sb[:, cc, msl], T1T_sb[:, cc, :],
                                     start=(cc == 0), stop=(cc == CT - 1))
                o = stage.tile([P, KL], F32, tag="outstage")
                nc.scalar.activation(o, ps, Identity,
                                     bias=b_gcn_sb[:, m:m + 1], scale=1.0)
                nc.sync.dma_start(out=out[m * P:(m + 1) * P, :], in_=o)

    nc.finalize()
    return nc


_NC_CACHE = None


def _get_nc():
    global _NC_CACHE
    if _NC_CACHE is None:
        _NC_CACHE = build()
    return _NC_CACHE


def _bf(a):
    return np.ascontiguousarray(a).astype(ml_dtypes.bfloat16)


def kernel(vc, vm, W_ak, b_ak, W_c, b_c, W_mad, b_mad, W_gcn, b_gcn):
    nc = _get_nc()

    W_akT = _bf(np.asarray(W_ak).T)
    W_cT = _bf(np.asarray(W_c).T)
    W_madT = _bf(np.asarray(W_mad).T)
    W_gcn_b = _bf(np.asarray(W_gcn))
    b_ak_t = np.ascontiguousarray(
        np.asarray(b_ak, np.float32).reshape(KT, P).T)
    b_cB = np.ascontiguousarray(
        np.tile(np.asarray(b_c, np.float32)[None, :], (P, 1)))
    b_mad_t = np.ascontiguousarray(
        np.asarray(b_mad, np.float32).reshape(CT, P).T)
    b_gcn_t = np.ascontiguousarray(
        np.asarray(b_gcn, np.float32).reshape(CT, P).T)

    vc0 = np.asarray(vc)[0]
    vm0 = np.asarray(vm)[0]

    in_maps = []
    for i in range(NCORES):
        cols = slice(i * KL, (i + 1) * KL)
        in_maps.append({
            "vc_i": _bf(vc0[:, cols]),
            "vm_i": _bf(vm0[:, cols]),
            "W_akT": W_akT,
            "W_cT": W_cT,
            "W_madT": W_madT,
            "W_gcn": W_gcn_b,
            "b_ak_t": b_ak_t,
            "b_cB": b_cB,
            "b_mad_t": b_mad_t,
            "b_gcn_t": b_gcn_t,
        })

    res = bass_utils.run_bass_kernel_spmd(nc, in_maps,
                                          core_ids=list(range(NCORES)))
    out = np.concatenate([np.asarray(res.results[i]["out"])
                          for i in range(NCORES)], axis=1)
    return out[None].astype(np.float32)


# revision 8
# speedup vs baseline: 1.1163x; 1.1163x over previous
"""Distributed Trainium2 Bass kernel for nn_AGCN (gnn_message_passing).

Reference computation (B=1, CHNN=1024, K=L=2048):
    vcw  = softmax_k(W_ak @ vc + b_ak)            # (K, L) assignment weights
    vmr  = relu(W_c @ vm + b_c)                   # (C, K)
    vma  = vmr @ vcw                              # (C, L)
    vmad = W_mad @ vma + b_mad                    # (C, L)
    A    = vmad^T @ vmad                          # (K, L) gram (symmetric)
    x    = vmr^T @ W_gcn + b_gcn                  # (K, C)
    out  = (softmax_rows(A) @ x)^T                # (C, L)

Distribution: position (node) sharding.  Core i owns 256 of the 2048 node
columns.  Everything is local except two bf16 AllGathers: vmrT (2048,1024)
and vmad (1024,256 per-core shards).  The final GEMM is refactored as
out = (A_sm @ vmr^T) @ W_gcn + b_gcn (rows of A_sm sum to 1, so the bias
add is exact), which removes a third all-gather.

All matmuls run in bf16 (fp32 PSUM accumulation).  Numerically validated:
full-bf16 pipeline gives ~9e-4 max rel err vs the fp64 reference (the A
softmax is near-uniform, so gram noise averages out).  Softmaxes skip
max-subtraction (z in ±3.4, A in [16.8, 17.2] for this input scale; exp
stays well inside fp32/bf16 range).
"""

import numpy as np
import ml_dtypes

import concourse.bass as bass
import concourse.mybir as mybir
import concourse.tile as tile
from concourse import bacc
from concourse import bass_utils

P = 128            # partitions
C = 1024           # channels (8 tiles)
K = 2048           # nodes (16 tiles)
NCORES = 8
KL = K // NCORES   # 256 local node columns per core
CT = C // P        # 8
KT = K // P        # 16
KLT = KL // P      # 2

BF = mybir.dt.bfloat16
F32 = mybir.dt.float32
RG = [list(range(NCORES))]

Exp = mybir.ActivationFunctionType.Exp
Identity = mybir.ActivationFunctionType.Identity


def build():
    nc = bacc.Bacc("TRN2", target_bir_lowering=False, debug=False,
                   num_devices=NCORES)

    # ---- kernel I/O (per-core) ----
    vc_i = nc.dram_tensor("vc_i", [C, KL], BF, kind="ExternalInput").ap()
    vm_i = nc.dram_tensor("vm_i", [C, KL], BF, kind="ExternalInput").ap()
    W_akT = nc.dram_tensor("W_akT", [C, K], BF, kind="ExternalInput").ap()
    W_cT = nc.dram_tensor("W_cT", [C, C], BF, kind="ExternalInput").ap()
    W_madT = nc.dram_tensor("W_madT", [C, C], BF, kind="ExternalInput").ap()
    W_gcn = nc.dram_tensor("W_gcn", [C, C], BF, kind="ExternalInput").ap()
    b_ak_t = nc.dram_tensor("b_ak_t", [P, KT], F32, kind="ExternalInput").ap()
    b_cB = nc.dram_tensor("b_cB", [P, C], F32, kind="ExternalInput").ap()
    b_mad_t = nc.dram_tensor("b_mad_t", [P, CT], F32, kind="ExternalInput").ap()
    b_gcn_t = nc.dram_tensor("b_gcn_t", [P, CT], F32, kind="ExternalInput").ap()
    out = nc.dram_tensor("out", [C, KL], F32, kind="ExternalOutput").ap()

    with tile.TileContext(nc) as tc:
        with (
            tc.tile_pool(name="const", bufs=1) as const,
            tc.tile_pool(name="stage", bufs=3) as stage,
            tc.tile_pool(name="psum", bufs=4, space="PSUM") as psum_pool,
            tc.tile_pool(name="cspsum", bufs=1, space="PSUM") as cs_pool,
            tc.tile_pool(name="dram", bufs=1, space="DRAM") as dram,
        ):
            # ---- persistent SBUF tensors ----
            vm_sb = const.tile([P, CT, KL], BF)       # vm   [p, ct, kl]
            vc_sb = const.tile([P, CT, KL], BF)
            WcT_sb = const.tile([P, CT, C], BF)       # W_c^T rows
            WakT_sb = const.tile([P, CT, K], BF)
            WmadT_sb = const.tile([P, CT, C], BF)
            Wgcn_sb = const.tile([P, CT, C], BF)
            vmrT_full = const.tile([P, KT, C], BF)    # vmr^T[kt*128+p, c]
            # vmad_full split by AG2 chunk: A = even global k-tiles (2s),
            # B = odd (2s+1); [p, ct, s, kl] = vmad[ct*128+p, s*256 + q*128 + kl]
            vmad_fullA = const.tile([P, CT, NCORES, P], BF)
            vmad_fullB = const.tile([P, CT, NCORES, P], BF)
            exp_sb = const.tile([P, KT, KL], BF)      # expz then expA (reused)
            expA_sb = exp_sb                          # alias for readability
            vma_sb = const.tile([P, CT, KL], BF)
            vmad_i_sb = const.tile([P, CT, KL], BF)
            T1T_sb = const.tile([P, CT, KL], BF)
            b_ak_sb = const.tile([P, KT], F32)
            b_cB_sb = const.tile([P, C], F32)
            b_mad_sb = const.tile([P, CT], F32)
            b_gcn_sb = const.tile([P, CT], F32)
            onesm = const.tile([P, P], BF)            # 128x128 ones
            recipL = const.tile([P, KL], F32)         # 1/colsum(expz), bcast rows
            recipR = const.tile([P, KL], F32)

            nc.any.memset(onesm, 1.0)

            # ---- input DMA loads: S1's dependencies first ----
            nc.sync.dma_start(out=b_cB_sb, in_=b_cB)
            for ct in range(CT):
                rows = slice(ct * P, (ct + 1) * P)
                nc.sync.dma_start(out=vm_sb[:, ct, :], in_=vm_i[rows, :])
                nc.sync.dma_start(out=WcT_sb[:, ct, :], in_=W_cT[rows, :])
            nc.sync.dma_start(out=b_ak_sb, in_=b_ak_t)
            nc.sync.dma_start(out=b_mad_sb, in_=b_mad_t)
            nc.sync.dma_start(out=b_gcn_sb, in_=b_gcn_t)

            # ---- collective bounce buffers ----
            ag1_in = dram.tile([KL, C], BF)
            ag1_out = dram.tile([K, C], BF, addr_space="Shared")
            ag2a_in = dram.tile([C, KL // 2], BF)
            ag2a_out = dram.tile([NCORES, C, KL // 2], BF, addr_space="Shared")
            ag2b_in = dram.tile([C, KL // 2], BF)
            ag2b_out = dram.tile([NCORES, C, KL // 2], BF, addr_space="Shared")

            # ================= S1: vmrT_i = relu(vm_i^T W_c^T + b_c) ======
            # out (KL=256 rows, C=1024): 2 k-slices x 2 n-chunks of 512
            with nc.named_scope("S1_vmrT"):
                for kt in range(KLT):
                    ksl = slice(kt * P, (kt + 1) * P)
                    for n in range(2):
                        nsl = slice(n * 512, (n + 1) * 512)
                        ps = psum_pool.tile([P, 512], F32, tag="ps512", bufs=2)
                        for cc in range(CT):
                            nc.tensor.matmul(ps, vm_sb[:, cc, ksl],
                                             WcT_sb[:, cc, nsl],
                                             start=(cc == 0), stop=(cc == CT - 1))
                        tmp = stage.tile([P, 512], F32, tag="s1tmp")
                        nc.vector.tensor_add(tmp, ps, b_cB_sb[:, nsl])
                        relu = stage.tile([P, 512], BF, tag="s1relu")
                        nc.vector.tensor_scalar_max(relu, tmp, 0.0)
                        nc.sync.dma_start(out=ag1_in[ksl, nsl], in_=relu)

            # ================= AG1: all-gather vmrT ========================
            nc.gpsimd.collective_compute(
                "AllGather", mybir.AluOpType.bypass, replica_groups=RG,
                ins=[ag1_in.opt()], outs=[ag1_out.opt()],
            )

            # remaining input loads (used by S3/S5/S9) come after the AG1
            # trigger chain so they don't delay it in the DMA queues
            for ct in range(CT):
                rows = slice(ct * P, (ct + 1) * P)
                nc.sync.dma_start(out=vc_sb[:, ct, :], in_=vc_i[rows, :])
            for ct in range(CT):
                rows = slice(ct * P, (ct + 1) * P)
                nc.sync.dma_start(out=WakT_sb[:, ct, :], in_=W_akT[rows, :])
            for ct in range(CT):
                rows = slice(ct * P, (ct + 1) * P)
                nc.sync.dma_start(out=WmadT_sb[:, ct, :], in_=W_madT[rows, :])
            for ct in range(CT):
                rows = slice(ct * P, (ct + 1) * P)
                nc.sync.dma_start(out=Wgcn_sb[:, ct, :], in_=W_gcn[rows, :])

            # ================= S3: expz = exp(W_ak vc + b_ak) ==============
            # out (K=2048 rows, KL=256): 16 k-tiles; colsum on all partitions
            cs1 = cs_pool.tile([P, KL], F32, tag="cs")
            with nc.named_scope("S3_expz"):
                for kt in range(KT):
                    ksl = slice(kt * P, (kt + 1) * P)
                    ps = psum_pool.tile([P, KL], F32, tag="ps256", bufs=5)
                    for cc in range(CT):
                        nc.tensor.matmul(ps, WakT_sb[:, cc, ksl],
                                         vc_sb[:, cc, :],
                                         start=(cc == 0), stop=(cc == CT - 1))
                    nc.scalar.activation(exp_sb[:, kt, :], ps, Exp,
                                         bias=b_ak_sb[:, kt:kt + 1], scale=1.0)
                    nc.tensor.matmul(cs1, onesm, exp_sb[:, kt, :],
                                     start=(kt == 0), stop=(kt == KT - 1))
                nc.vector.reciprocal(recipL, cs1)

            # ---- load gathered vmrT into SBUF ----
            for kt in range(KT):
                rows = slice(kt * P, (kt + 1) * P)
                nc.sync.dma_start(out=vmrT_full[:, kt, :], in_=ag1_out[rows, :])

            # ================= S4: vma = (vmr @ expz) * recipL =============
            with nc.named_scope("S4_vma"):
                for m in range(CT):
                    msl = slice(m * P, (m + 1) * P)
                    ps = psum_pool.tile([P, KL], F32, tag="ps256", bufs=5)
                    for kt in range(KT):
                        nc.tensor.matmul(ps, vmrT_full[:, kt, msl],
                                         exp_sb[:, kt, :],
                                         start=(kt == 0), stop=(kt == KT - 1))
                    nc.vector.tensor_mul(vma_sb[:, m, :], ps, recipL)

            # ================= S5: vmad = W_mad vma + b_mad ================
            with nc.named_scope("S5_vmad"):
                for m in range(CT):
                    msl = slice(m * P, (m + 1) * P)
                    ps = psum_pool.tile([P, KL], F32, tag="ps256", bufs=5)
                    for cc in range(CT):
                        nc.tensor.matmul(ps, WmadT_sb[:, cc, msl],
                                         vma_sb[:, cc, :],
                                         start=(cc == 0), stop=(cc == CT - 1))
                    nc.scalar.activation(vmad_i_sb[:, m, :], ps, Identity,
                                         bias=b_mad_sb[:, m:m + 1], scale=1.0)
                    nc.sync.dma_start(out=ag2a_in[msl, :],
                                      in_=vmad_i_sb[:, m, :KL // 2])
                    nc.sync.dma_start(out=ag2b_in[msl, :],
                                      in_=vmad_i_sb[:, m, KL // 2:])

            # ========== AG2 (2 k-chunks): all-gather vmad ==================
            # chunk a = each rank's local columns 0:128  -> even global k-tiles
            # chunk b = each rank's local columns 128:256 -> odd global k-tiles
            nc.gpsimd.collective_compute(
                "AllGather", mybir.AluOpType.bypass, replica_groups=RG,
                ins=[ag2a_in.opt()], outs=[ag2a_out.opt()],
            )
            nc.gpsimd.collective_compute(
                "AllGather", mybir.AluOpType.bypass, replica_groups=RG,
                ins=[ag2b_in.opt()], outs=[ag2b_out.opt()],
            )
            # reassemble: vmad_fullA[p, ct, s, kl] = vmad[c, s*256+kl], kl<128
            for ct in range(CT):
                rows = slice(ct * P, (ct + 1) * P)
                nc.sync.dma_start(
                    out=vmad_fullA[:, ct, :, :],
                    in_=ag2a_out[:, rows, :].rearrange("s p k -> p s k"),
                )
            for ct in range(CT):
                rows = slice(ct * P, (ct + 1) * P)
                nc.sync.dma_start(
                    out=vmad_fullB[:, ct, :, :],
                    in_=ag2b_out[:, rows, :].rearrange("s p k -> p s k"),
                )

            # ================= S7: expA = exp(vmad_full^T vmad_i) ==========
            # even k-tiles (chunk a) first, then odd (chunk b)
            cs2 = cs_pool.tile([P, KL], F32, tag="cs")
            kt_order = [2 * s for s in range(NCORES)] + \
                       [2 * s + 1 for s in range(NCORES)]
            with nc.named_scope("S7_expA"):
                for idx, kt in enumerate(kt_order):
                    s, q = kt // 2, kt % 2
                    src = vmad_fullA if q == 0 else vmad_fullB
                    ps = psum_pool.tile([P, KL], F32, tag="ps256", bufs=5)
                    for cc in range(CT):
                        nc.tensor.matmul(ps, src[:, cc, s, :],
                                         vmad_i_sb[:, cc, :],
                                         start=(cc == 0), stop=(cc == CT - 1))
                    nc.scalar.activation(expA_sb[:, kt, :], ps, Exp)
                    nc.tensor.matmul(cs2, onesm, expA_sb[:, kt, :],
                                     start=(idx == 0), stop=(idx == KT - 1))
                nc.vector.reciprocal(recipR, cs2)

            # ================= S8: T1T = (vmr @ expA) * recipR =============
            with nc.named_scope("S8_T1T"):
                for m in range(CT):
                    msl = slice(m * P, (m + 1) * P)
                    ps = psum_pool.tile([P, KL], F32, tag="ps256", bufs=5)
                    for kt in range(KT):
                        nc.tensor.matmul(ps, vmrT_full[:, kt, msl],
                                         expA_sb[:, kt, :],
                                         start=(kt == 0), stop=(kt == KT - 1))
                    nc.vector.tensor_mul(T1T_sb[:, m, :], ps, recipR)

            # ================= S9: out = W_gcn^T T1T + b_gcn ===============
            with nc.named_scope("S9_out"):
                for m in range(CT):
                    msl = slice(m * P, (m + 1) * P)
                    ps = psum_pool.tile([P, KL], F32, tag="ps256", bufs=5)
                    for cc in range(CT):
                        nc.tensor.matmul(ps, Wgcn_sb[:, cc, msl],
                                         T1T_sb[:, cc, :],
                                         start=(cc == 0), stop=(cc == CT - 1))
                    o = stage.tile([P, KL], F32, tag="outstage")
                    nc.scalar.activation(o, ps, Identity,
                                         bias=b_gcn_sb[:, m:m + 1], scale=1.0)
                    nc.sync.dma_start(out=out[m * P:(m + 1) * P, :], in_=o)

    nc.finalize()
    return nc


_NC_CACHE = None


def _get_nc():
    global _NC_CACHE
    if _NC_CACHE is None:
        _NC_CACHE = build()
    return _NC_CACHE


def _bf(a):
    return np.ascontiguousarray(a).astype(ml_dtypes.bfloat16)


def kernel(vc, vm, W_ak, b_ak, W_c, b_c, W_mad, b_mad, W_gcn, b_gcn):
    nc = _get_nc()

    W_akT = _bf(np.asarray(W_ak).T)
    W_cT = _bf(np.asarray(W_c).T)
    W_madT = _bf(np.asarray(W_mad).T)
    W_gcn_b = _bf(np.asarray(W_gcn))
    b_ak_t = np.ascontiguousarray(
        np.asarray(b_ak, np.float32).reshape(KT, P).T)
    b_cB = np.ascontiguousarray(
        np.tile(np.asarray(b_c, np.float32)[None, :], (P, 1)))
    b_mad_t = np.ascontiguousarray(
        np.asarray(b_mad, np.float32).reshape(CT, P).T)
    b_gcn_t = np.ascontiguousarray(
        np.asarray(b_gcn, np.float32).reshape(CT, P).T)

    vc0 = np.asarray(vc)[0]
    vm0 = np.asarray(vm)[0]

    in_maps = []
    for i in range(NCORES):
        cols = slice(i * KL, (i + 1) * KL)
        in_maps.append({
            "vc_i": _bf(vc0[:, cols]),
            "vm_i": _bf(vm0[:, cols]),
            "W_akT": W_akT,
            "W_cT": W_cT,
            "W_madT": W_madT,
            "W_gcn": W_gcn_b,
            "b_ak_t": b_ak_t,
            "b_cB": b_cB,
            "b_mad_t": b_mad_t,
            "b_gcn_t": b_gcn_t,
        })

    res = bass_utils.run_bass_kernel_spmd(nc, in_maps,
                                          core_ids=list(range(NCORES)))
    out = np.concatenate([np.asarray(res.results[i]["out"])
                          for i in range(NCORES)], axis=1)
    return out[None].astype(np.float32)
